# revision 11
# baseline (speedup 1.0000x reference)
"""BiLSTM-CRF forward + Viterbi decode on 8 Trainium2 NeuronCores.

Self-contained: hardcodes all shapes. Data-parallel over the batch dim:
8 cores x 4 sequences each. Per core, on device:
  - embedding gather (indirect DMA from the 100k x 256 table in DRAM)
  - input projections x @ W_ih^T + b (batched matmul, fp32)
  - fwd+bwd LSTM recurrence (gates-on-partitions layout, fp32)
  - output head -> CRF emission scores
  - Viterbi: forward max-plus scan + backward (beta) scan, path by
    argmax(alpha+beta) per step (equivalent to backpointer backtrace for
    unique maxima), score = max(alpha_T + trans[STOP]).

kernel(**inputs) takes the FULL unsharded inputs (numpy arrays, keys as in
setup_inputs) and returns (score [32] f32, path [32, 256] int32).
"""

import numpy as np

V, E, H, NK = 100000, 256, 256, 12
B, T_FULL = 32, 256
NCORES = 8
BL = B // NCORES          # 4 sequences per core
START, STOP = 9, 10
NEG = -10000.0

# PyTorch gate order i,f,g,o  ->  ours i,f,o,g (sigmoid block contiguous)
_GATE_PERM = np.concatenate(
    [np.arange(0, 512), np.arange(768, 1024), np.arange(512, 768)]
)


# ---------------------------------------------------------------- device code
def build_nc(T=T_FULL):
    import concourse.tile as tile
    from concourse import mybir, bacc
    from concourse.masks import make_identity

    TB = T * BL                      # time-major token count per core
    TP = min(128, T)                 # t-partition count for bulk viterbi ops
    R = T // TP                      # rows per partition in t-partitioned maps
    NTT = (TB + 511) // 512          # tb tiles of <=512 for the projections
    NTB = min(512, TB)
    NCH = max(1, TB // 128)          # gather chunks of 128 rows
    f32 = mybir.dt.float32
    i32 = mybir.dt.int32
    AX = mybir.AxisListType
    OP = mybir.AluOpType
    AF = mybir.ActivationFunctionType

    nc = bacc.Bacc(target_bir_lowering=False)

    # ---- DRAM inputs (per-core shards prepared on host)
    emb_d = nc.dram_tensor("emb", [V, E], f32, kind="ExternalInput")
    idx_d = nc.dram_tensor("idx", [128, NCH], i32, kind="ExternalInput")
    wih_d = nc.dram_tensor("wih", [128, 2, 2, 1024], f32, kind="ExternalInput")
    whh_d = nc.dram_tensor("whh", [128, 2, 2, 1024], f32, kind="ExternalInput")
    bias_d = nc.dram_tensor("bias", [128, 2, 8], f32, kind="ExternalInput")
    h0c0_d = nc.dram_tensor("h0c0", [128, 2, 2, 2, BL], f32, kind="ExternalInput")
    wout_d = nc.dram_tensor("wout", [128, 4, NK], f32, kind="ExternalInput")
    # transitions (+b_out folded), replicated per-partition: [128, n*12+p]
    trf_d = nc.dram_tensor("trf", [128, NK * NK], f32, kind="ExternalInput")
    trstop_d = nc.dram_tensor("trstop", [BL, NK], f32, kind="ExternalInput")
    fv0_d = nc.dram_tensor("fv0", [BL, NK], f32, kind="ExternalInput")
    iotar_d = nc.dram_tensor("iotar", [BL, NK], f32, kind="ExternalInput")

    # ---- DRAM outputs
    score_d = nc.dram_tensor("score", [BL, 1], f32, kind="ExternalOutput")
    path_d = nc.dram_tensor("path", [BL, T], i32, kind="ExternalOutput")

    # ---- DRAM intermediates
    fdram = nc.dram_tensor("fdram", [TB, NK], f32)
    ddram = nc.dram_tensor("ddram", [T, BL, NK * NK], f32)

    import concourse.bass as bass

    with tile.TileContext(nc) as tc:
        with tc.tile_pool(name="const", bufs=1) as cp, \
             tc.tile_pool(name="state", bufs=1) as stp, \
             tc.tile_pool(name="work", bufs=4) as wp, \
             tc.tile_pool(name="psA", bufs=2, space="PSUM") as psA, \
             tc.tile_pool(name="psG", bufs=2, space="PSUM") as psG:

            # ================= persistent loads =================
            idx_sb = cp.tile([128, NCH], i32)
            nc.sync.dma_start(idx_sb[:], idx_d[:])
            whh_sb = cp.tile([128, 2, 2, 1024], f32)
            nc.sync.dma_start(whh_sb[:], whh_d[:])
            bias_sb = cp.tile([128, 2, 8], f32)
            nc.sync.dma_start(bias_sb[:], bias_d[:])
            h0c0_sb = cp.tile([128, 2, 2, 2, BL], f32)
            nc.sync.dma_start(h0c0_sb[:], h0c0_d[:])
            wout_sb = cp.tile([128, 4, NK], f32)
            nc.sync.dma_start(wout_sb[:], wout_d[:])
            trf_sb = cp.tile([128, 1, 1, NK, NK], f32)
            nc.sync.dma_start(trf_sb[:, 0, 0, :, :],
                              trf_d.rearrange("p (n q) -> p n q", q=NK))
            iotar = cp.tile([BL, 1, NK], f32)
            nc.sync.dma_start(iotar[:, 0, :], iotar_d[:])

            H_hist = stp.tile([128, 2, 2, TB], f32)
            c_tiles = [stp.tile([128, 2, 2, BL], f32, name=f"cst{i}")
                       for i in range(2)]
            fvh = stp.tile([BL, T + 1, NK], f32)
            nc.sync.dma_start(fvh[:, 0, :], fv0_d[:])
            beh = stp.tile([BL, T + 1, NK], f32)
            nc.sync.dma_start(beh[:, T, :], trstop_d[:])

            with tc.tile_pool(name="apool", bufs=1) as ap:
                # A[p, d, m, tb] = (x @ Wih_d^T)[tb, 128m+p] + bias_d[128m+p]
                A = ap.tile([128, 2, 8, TB], f32)

                with tc.tile_pool(name="projw", bufs=1) as pp:
                    wih_sb = pp.tile([128, 2, 2, 1024], f32)
                    nc.sync.dma_start(wih_sb[:], wih_d[:])
                    ident = pp.tile([128, 128], f32)
                    make_identity(nc, ident[:])

                    # ---- embedding gather
                    x_g = pp.tile([128, NCH, E], f32)
                    for ch in range(NCH):
                        nc.gpsimd.indirect_dma_start(
                            out=x_g[:, ch, :],
                            out_offset=None,
                            in_=emb_d[:],
                            in_offset=bass.IndirectOffsetOnAxis(
                                ap=idx_sb[:, ch:ch + 1], axis=0
                            ),
                        )

                    # ---- transpose x -> xT [e, tb]
                    xT = pp.tile([128, 2, TB], f32)
                    for ch in range(NCH):
                        for kc in range(2):
                            pt = psA.tile([128, 128], f32, tag="tp", name="pt")
                            nc.tensor.transpose(
                                out=pt[:],
                                in_=x_g[:, ch, kc * 128:(kc + 1) * 128],
                                identity=ident[:],
                            )
                            nc.vector.tensor_copy(
                                xT[:, kc, ch * 128:(ch + 1) * 128], pt[:])

                    # ---- input projections
                    for d, nt in [(0, 0), (1, NTT - 1)] + [
                        (dd, tt) for tt in range(NTT) for dd in (0, 1)
                        if (dd, tt) not in ((0, 0), (1, NTT - 1))
                    ]:
                        n0 = nt * NTB
                        for m in range(8):
                            acc = psA.tile([128, NTB], f32, tag="pj", name="acc")
                            for kc in range(2):
                                nc.tensor.matmul(
                                    acc[:],
                                    wih_sb[:, d, kc, m * 128:(m + 1) * 128],
                                    xT[:, kc, n0:n0 + NTB],
                                    start=(kc == 0), stop=(kc == 1),
                                )
                            nc.scalar.add(A[:, d, m, n0:n0 + NTB], acc[:],
                                          bias_sb[:, d, m:m + 1])

                # ================= LSTM recurrence =================
                for t in range(T):
                    for d in range(2):
                        ta = t if d == 0 else T - 1 - t
                        if t == 0:
                            h_prev = h0c0_sb[:, 0, d, :, :]
                            c_prev = h0c0_sb[:, 1, d, :, :]
                        else:
                            pa = t - 1 if d == 0 else T - t
                            h_prev = H_hist[:, d, :, pa * BL:(pa + 1) * BL]
                            c_prev = c_tiles[(t - 1) % 2][:, d, :, :]
                        c_new = c_tiles[t % 2][:, d, :, :]

                        g_ps = psG.tile([128, 8, BL], f32, tag="g", name="g_ps")
                        for m in range(8):
                            for kc in range(2):
                                nc.tensor.matmul(
                                    g_ps[:, m, :],
                                    whh_sb[:, d, kc, m * 128:(m + 1) * 128],
                                    h_prev[:, kc, :],
                                    start=(m == 0 and kc == 0),
                                    stop=(m == 7 and kc == 1),
                                )
                        gs = wp.tile([128, 8, BL], f32, tag="gs", name="gs")
                        nc.vector.tensor_tensor(
                            out=gs[:], in0=g_ps[:],
                            in1=A[:, d, :, ta * BL:(ta + 1) * BL], op=OP.add)
                        ga = wp.tile([128, 8, BL], f32, tag="ga", name="ga")
                        nc.scalar.activation(ga[:, 0:6, :], gs[:, 0:6, :],
                                             AF.Sigmoid)
                        nc.scalar.activation(ga[:, 6:8, :], gs[:, 6:8, :], AF.Tanh)
                        p2 = wp.tile([128, 2, BL], f32, tag="p2", name="p2")
                        nc.vector.tensor_tensor(out=p2[:], in0=ga[:, 0:2, :],
                                                in1=ga[:, 6:8, :], op=OP.mult)
                        fc = wp.tile([128, 2, BL], f32, tag="fc", name="fc")
                        nc.vector.tensor_tensor(out=fc[:], in0=ga[:, 2:4, :],
                                                in1=c_prev, op=OP.mult)
                        nc.vector.tensor_tensor(out=c_new, in0=fc[:], in1=p2[:],
                                                op=OP.add)
                        tc_t = wp.tile([128, 2, BL], f32, tag="tc", name="tc_t")
                        nc.scalar.activation(tc_t[:], c_new, AF.Tanh)
                        nc.vector.tensor_tensor(
                            out=H_hist[:, d, :, ta * BL:(ta + 1) * BL],
                            in0=ga[:, 4:6, :], in1=tc_t[:], op=OP.mult,
                        )

            # ================= output head =================
            f_ps = psA.tile([128, TB // 128, NK], f32, tag="hd", name="f_ps")
            for mc in range(TB // 128):
                for c4 in range(4):
                    d, kc = divmod(c4, 2)
                    nc.tensor.matmul(
                        f_ps[:, mc, :],
                        H_hist[:, d, kc, mc * 128:(mc + 1) * 128],
                        wout_sb[:, c4, :],
                        start=(mc == 0 and c4 == 0),
                        stop=(mc == TB // 128 - 1 and c4 == 3),
                    )
            with tc.tile_pool(name="vit", bufs=1) as vp, \
                 tc.tile_pool(name="dbf", bufs=2) as dfp, \
                 tc.tile_pool(name="dbb", bufs=2) as dbp:
                feats_sb = vp.tile([128, TB // 128, NK], f32)
                nc.vector.tensor_copy(feats_sb[:], f_ps[:])
                nc.sync.dma_start(
                    fdram.rearrange("(mc p) k -> p mc k", p=128), feats_sb[:])

                # ---- D_t[n, p] = trans[n, p] + b_out[n] + feat_t[n]
                f128 = vp.tile([TP, R, BL, NK], f32)
                nc.sync.dma_start(
                    f128[:],
                    fdram.rearrange("(r p b) k -> p r b k", p=TP, b=BL))
                d128 = vp.tile([TP, R, BL, NK, NK], f32)
                nc.vector.tensor_tensor(
                    out=d128[:],
                    in0=f128[:].broadcast_to([TP, R, BL, NK, NK]),
                    in1=trf_sb[:TP].broadcast_to([TP, R, BL, NK, NK]),
                    op=OP.add,
                )
                nc.sync.dma_start(
                    ddram.rearrange("(r p) b nq -> p r b nq", p=TP),
                    d128[:].rearrange("p r b n q -> p r b (n q)"),
                )

                # ---- forward (alpha) and backward (beta) max-plus scans
                CH = min(16, T)
                NCHV = T // CH
                for ci in range(NCHV):
                    dbf = dfp.tile([BL, CH, NK, NK], f32, tag="dbf", name="dbf")
                    nc.sync.dma_start(
                        dbf[:],
                        ddram[ci * CH:(ci + 1) * CH].rearrange(
                            "t b (n q) -> b t n q", q=NK),
                    )
                    dbb = dbp.tile([BL, CH, NK, NK], f32, tag="dbb", name="dbb")
                    cib = NCHV - 1 - ci
                    nc.sync.dma_start(
                        dbb[:],
                        ddram[cib * CH:(cib + 1) * CH].rearrange(
                            "t b (n q) -> b t n q", q=NK),
                    )
                    for j in range(CH):
                        t = ci * CH + j
                        sc = wp.tile([BL, NK, NK], f32, tag="sc", name="sc")
                        nc.vector.tensor_tensor(
                            out=sc[:], in0=dbf[:, j, :, :],
                            in1=fvh[:, t:t + 1, :].broadcast_to([BL, NK, NK]),
                            op=OP.add,
                        )
                        nc.vector.tensor_reduce(
                            out=fvh[:, t + 1, :], in_=sc[:], axis=AX.X, op=OP.max)

                        tb_ = cib * CH + (CH - 1 - j)
                        sb_ = wp.tile([BL, NK, NK], f32, tag="sb", name="sb_")
                        nc.vector.tensor_tensor(
                            out=sb_[:],
                            in0=dbb[:, CH - 1 - j, :, :].rearrange(
                                "b n q -> b q n"),
                            in1=beh[:, tb_ + 1:tb_ + 2, :].broadcast_to(
                                [BL, NK, NK]),
                            op=OP.add,
                        )
                        nc.vector.tensor_reduce(
                            out=beh[:, tb_, :], in_=sb_[:], axis=AX.X, op=OP.max)

                # ---- path = argmax_n(alpha_{t+1} + beta_{t+1}), first-max ties
                slab = vp.tile([BL, T, NK], f32)
                nc.vector.tensor_tensor(out=slab[:], in0=fvh[:, 1:T + 1, :],
                                        in1=beh[:, 1:T + 1, :], op=OP.add)
                mslab = vp.tile([BL, T, 1], f32)
                nc.vector.tensor_reduce(out=mslab[:, :, 0], in_=slab[:],
                                        axis=AX.X, op=OP.max)
                oh = vp.tile([BL, T, NK], f32)
                nc.vector.tensor_tensor(
                    out=oh[:], in0=slab[:],
                    in1=mslab[:].broadcast_to([BL, T, NK]),
                    op=OP.is_equal,
                )
                pr = vp.tile([BL, T, NK], f32)
                nc.vector.tensor_tensor(
                    out=pr[:], in0=oh[:],
                    in1=iotar[:].broadcast_to([BL, T, NK]),
                    op=OP.mult,
                )
                prm = vp.tile([BL, T], f32)
                nc.vector.tensor_reduce(out=prm[:], in_=pr[:], axis=AX.X,
                                        op=OP.max)
                pathf = vp.tile([BL, T], f32)
                nc.vector.tensor_scalar(out=pathf[:], in0=prm[:], scalar1=-1.0,
                                        scalar2=float(NK - 1), op0=OP.mult,
                                        op1=OP.add)
                pathi = vp.tile([BL, T], i32)
                nc.vector.tensor_copy(pathi[:], pathf[:])
                nc.sync.dma_start(path_d[:], pathi[:])

                sco = vp.tile([BL, 1], f32)
                nc.vector.tensor_copy(sco[:], mslab[:, T - 1, :])
                nc.sync.dma_start(score_d[:], sco[:])

    nc.compile()
    return nc


# ---------------------------------------------------------------- host prep
def prep_shards(inputs, T=T_FULL):
    """Returns list of 8 in_maps (numpy) for run_bass_kernel_spmd."""
    f = lambda k: np.asarray(inputs[k], dtype=np.float32)
    sent = np.asarray(inputs["sentence"]).astype(np.int32)[:, :T]
    emb = f("emb")
    P = _GATE_PERM

    def wih_pack(Wf, Wb):
        # out[p, d, kc, g] = W_d[P[g], 128*kc + p]
        out = np.empty((128, 2, 2, 1024), np.float32)
        for d, W in enumerate((Wf, Wb)):
            Wp = W[P, :]                       # [1024, 256]
            out[:, d] = Wp.T.reshape(2, 128, 1024).transpose(1, 0, 2)
        return np.ascontiguousarray(out)

    wih = wih_pack(f("W_ih_f"), f("W_ih_b"))
    whh = wih_pack(f("W_hh_f"), f("W_hh_b"))
    bias = np.empty((128, 2, 8), np.float32)
    for d, (bi, bh) in enumerate((("b_ih_f", "b_hh_f"), ("b_ih_b", "b_hh_b"))):
        bb = (f(bi) + f(bh))[P]
        bias[:, d, :] = bb.reshape(8, 128).T
    wout = np.empty((128, 4, NK), np.float32)
    Wo = f("W_out")                             # [12, 512]
    for c4 in range(4):
        wout[:, c4, :] = Wo[:, c4 * 128:(c4 + 1) * 128].T
    b_out = f("b_out")
    trans = f("transitions")
    trf = (trans + b_out[:, None]).reshape(-1)          # [144], n-major
    trf128 = np.tile(trf[None, :], (128, 1)).astype(np.float32)
    trstop = np.tile(trans[STOP][None, :], (BL, 1)).astype(np.float32)
    fv0 = np.full((BL, NK), NEG, np.float32)
    fv0[:, START] = 0.0
    iotar = np.tile((NK - 1 - np.arange(NK, dtype=np.float32))[None, :], (BL, 1))
    h0 = f("h0")
    c0 = f("c0")

    TB = T * BL
    NCH = max(1, TB // 128)
    maps = []
    for c in range(NCORES):
        bs = slice(c * BL, (c + 1) * BL)
        flat = sent[bs].T.reshape(-1)                    # tb = t*BL + b
        idx = flat.reshape(NCH, 128).T.copy().astype(np.int32)
        h0c0 = np.empty((128, 2, 2, 2, BL), np.float32)
        for hc, arr in enumerate((h0, c0)):
            for d in range(2):
                h_t = arr[d, bs, :].T                    # [256, BL]
                h0c0[:, hc, d, :, :] = h_t.reshape(2, 128, BL).transpose(1, 0, 2)
        maps.append({
            "emb": emb, "idx": idx, "wih": wih, "whh": whh, "bias": bias,
            "h0c0": h0c0, "wout": wout, "trf": trf128, "trstop": trstop,
            "fv0": fv0, "iotar": iotar,
        })
    return maps


_NC_CACHE = {}


def kernel(**inputs):
    from concourse.bass_utils import run_bass_kernel_spmd
    if T_FULL not in _NC_CACHE:
        _NC_CACHE[T_FULL] = build_nc(T_FULL)
    nc = _NC_CACHE[T_FULL]
    maps = prep_shards(inputs, T_FULL)
    res = run_bass_kernel_spmd(nc, maps, list(range(NCORES)))
    score = np.concatenate([r["score"][:, 0] for r in res.results])
    path = np.concatenate([r["path"] for r in res.results]).astype(np.int32)
    return score.astype(np.float32), path
